# revision 11
# baseline (speedup 1.0000x reference)
"""Trainium2 Bass kernel for nn_Attention_85186381348942.

2D self-attention block: x [2, 512, 64, 64], 8 heads x 64 dim, n = 4096 tokens.
  qkv = w_qkv @ x ; per head: S = (q*scale)^T k ; P = exp(S) (softmax without
  max-subtraction -- logits are small); out = (P/Z) @ v ; y = w_out @ out + b.

Sharding: 8 cores = (batch b in {0,1}) x (head-pair hp in {0..3}); each core
computes 2 heads of one batch and the partial output projection for its head
slice. Host sums the 4 partials per batch and adds bias.

Design (v2): the exp of the 33.5M scores/core is the bottleneck; it is split
across two engines:
 - ACT: direct exp, PSUM fp32 -> SBUF fp16, on ~20 of every 32 j-steps.
 - DVE: one-instruction Schraudolph on the rest: int16(s*2^10/ln2 + b)
   bitcast to fp16 is a ~1.8%-rms exp approximation; softmax normalization
   cancels most of it (end-to-end rel err ~7e-3, gate is 2e-2).
P is fp16 [j=128, A 512 | B 512].  The PV matmul is FLIPPED: per j-step,
8 small matmuls out[i-chunk 128, 65] with lhsT = P chunk (stationary) and
rhs = v' [128, 65] moving (65th column = ones -> Z); 65 moving rows instead
of 512 halves PE busy vs the unflipped form (HW-verified ~209ns/step).
PV accumulators are packed 4-per-PSUM-bank: each bank's first group at j=0
uses start=True (lazy-zeroes the bank); the rest ride start=False on the
pending-zero bytes.
Epilogue per i-block, spread over the next block's j-steps: reciprocal of
the Z columns, per-partition-scalar normalize (out[i,d] layout makes 1/Z a
per-partition scalar), DMA-transpose [128 i, 128 hd] -> [128 hd, 128 i]
(XBAR, on otherwise-idle DMA), then 4 full-K projection matmuls + stores.

PSUM map (8 banks): scores 2 x [128,1024] (tags s) = 4 banks; 4 single-bank
tags v00/v10/v01/v11: PV accumulators of the current i-block parity, while
the opposite parity's banks cycle through epilogue-read -> projection ->
qk-prefetch (and vt/k-stream during ib0).
"""

import numpy as np
import ml_dtypes

import concourse.bass as bass
import concourse.tile as tile
from concourse import bacc, mybir
from concourse.bass_utils import run_bass_kernel_spmd

BF16 = mybir.dt.bfloat16
F16 = mybir.dt.float16
F32 = mybir.dt.float32
I16 = mybir.dt.int16
AF = mybir.ActivationFunctionType
ALU = mybir.AluOpType

HEADS = 8
DIM_HEAD = 64
DIM = 512
N = 4096
N_CORES = 8
NB = 8    # i-blocks
JB = 32   # j-steps per i-block
WI = 512  # i-block width

# fp16 Schraudolph: exp(x) ~ bitcast_i16(int16(x * 2^10/ln2 + (15<<10) - C))
A16 = float(2**10 / np.log(2))
B16 = float(15 * 2**10 - 0.045 * 2**10)

# j-steps whose exp runs on DVE (the rest run on ACT)
DVE_J = frozenset({1, 7, 8, 10, 13, 15, 17, 20, 22, 24, 26, 28, 30})
DVE_J_IB0 = frozenset({5, 7, 10, 13, 15, 18, 21, 23, 26})


def build_program(repeats: int = 1, small_out: bool = False) -> bass.Bass:
    nc = bacc.Bacc(None, target_bir_lowering=False, num_devices=N_CORES)

    x = nc.dram_tensor("x", [DIM, N], BF16, kind="ExternalInput")
    wqk = nc.dram_tensor("wqk", [DIM, 256], BF16, kind="ExternalInput")
    wv = nc.dram_tensor("wv", [DIM, 128], BF16, kind="ExternalInput")
    wo2 = nc.dram_tensor("wo2", [128, 4, 128], BF16, kind="ExternalInput")
    if small_out:
        out = nc.dram_tensor("out", [4, 128, WI], F32, kind="ExternalOutput")
        out_r = out
    else:
        out = nc.dram_tensor("out", [DIM, N], F32, kind="ExternalOutput")
        out_r = out.rearrange("(o p) n -> o p n", p=128)

    x_r = x.rearrange("(o p) n -> p o n", p=128)
    wqk_r = wqk.rearrange("(o p) m -> p o m", p=128)
    wv_r = wv.rearrange("(o p) m -> p o m", p=128)

    with tile.TileContext(nc) as tc:
        with (
            tc.tile_pool(name="singles", bufs=1) as singles,
            tc.tile_pool(name="pp", bufs=1) as pp,      # P tiles (int16)
            tc.tile_pool(name="sbsm", bufs=2) as sbsm,  # small sbuf temps
            tc.tile_pool(name="stg", bufs=4) as stg,    # output staging
            tc.tile_pool(name="ps_s", bufs=2, space="PSUM") as ps_s,  # scores
            tc.tile_pool(name="ps_v", bufs=1, space="PSUM") as ps_v,  # pv/misc
        ):
            # ---- input loads: wqk first (it gates k0), then x block 0 at
            # o-chunk granularity so the k0 matmuls start early; bulk after
            wqk_sb = singles.tile([128, 4, 256], BF16)
            nc.sync.dma_start(wqk_sb[:], wqk_r)
            x_sb = singles.tile([128, 4, N], BF16)
            for o in range(4):
                nc.sync.dma_start(x_sb[:, o, 0:WI], x_r[:, o, 0:WI])
            for nb0 in range(1, 3):
                nc.sync.dma_start(
                    x_sb[:, :, nb0 * WI:(nb0 + 1) * WI],
                    x_r[:, :, nb0 * WI:(nb0 + 1) * WI])
            nc.sync.dma_start(x_sb[:, :, 3 * WI:], x_r[:, :, 3 * WI:])
            wv_sb = singles.tile([128, 4, 128], BF16)
            nc.gpsimd.dma_start(wv_sb[:], wv_r)
            wo2_sb = singles.tile([128, 4, 128], BF16)
            nc.gpsimd.dma_start(wo2_sb[:], wo2[:])

            q_sb = singles.tile([128, N], BF16)
            k_sb = singles.tile([128, N], BF16)
            # v' per j-chunk: [j-part, chunk, h*65 + d]; cols 64/129 = ones
            v2 = singles.tile([128, JB, 130], F16)
            nc.vector.memset(v2[:], 1.0)
            # dummy exp: pull the ACT table load into the x-DMA window
            warm = singles.tile([1, 8], F32)
            nc.vector.memset(warm[:], 0.0)
            nc.scalar.activation(warm[:], warm[:], AF.Exp)

            def emit_qk_block(m, dst, nb, tag, ps=None, o_only=None,
                              copy_on_act=False):
                """dst[:, nb] = wqk[:, m-chunk]^T @ x[:, nb-block]."""
                if ps is None:
                    ps = ps_v.tile([128, WI], F32, tag=tag, name="qk_ps")
                chunks = range(4) if o_only is None else [o_only]
                for o in chunks:
                    nc.tensor.matmul(
                        ps[:],
                        lhsT=wqk_sb[:, o, m * 128:(m + 1) * 128],
                        rhs=x_sb[:, o, nb * WI:(nb + 1) * WI],
                        start=(o == 0),
                        stop=(o == 3),
                        skip_group_check=True,
                    )
                if o_only in (None, 3):
                    if copy_on_act:
                        # Copy rides in the exp_and_others ACT table: no
                        # table reload between it and the Exp activations
                        nc.scalar.activation(
                            dst[:, nb * WI:(nb + 1) * WI], ps[:], AF.Copy)
                    else:
                        nc.vector.tensor_copy(
                            dst[:, nb * WI:(nb + 1) * WI], ps[:])
                return ps

            def emit_vt_chunk(t):
                """v2 chunk t (both heads): vT[n, hd] = x^T @ wv."""
                ps = ps_v.tile([128, 128], F32, tag="v01", name="vt_ps")
                for o in range(4):
                    nc.tensor.matmul(
                        ps[:],
                        lhsT=x_sb[:, o, t * 128:(t + 1) * 128],
                        rhs=wv_sb[:, o, :],
                        start=(o == 0),
                        stop=(o == 3),
                    )
                # strided copy into [d 0:64] and [65:129] (ones cols kept)
                nc.vector.tensor_copy(
                    v2[:, t, 0:130].rearrange("p (h d) -> p h d", h=2)[:, :, 0:64],
                    ps[:].rearrange("p (h d) -> p h d", h=2),
                )

            def emit_scores(ib, j):
                i0 = ib * WI
                j0 = j * 128
                s_ps = ps_s.tile([128, 2 * WI], F32, tag="s", name="s_ps")
                nc.tensor.matmul(
                    s_ps[:, 0:WI],
                    lhsT=k_sb[0:64, j0:j0 + 128],
                    rhs=q_sb[0:64, i0:i0 + WI],
                    start=True, stop=True,
                )
                nc.tensor.matmul(
                    s_ps[:, WI:2 * WI],
                    lhsT=k_sb[64:128, j0:j0 + 128],
                    rhs=q_sb[64:128, i0:i0 + WI],
                    start=True, stop=True,
                )
                return s_ps

            def emit_exp(s_ps, p_t, on_dve):
                if on_dve:
                    nc.vector.tensor_scalar(
                        p_t[:], s_ps[:], A16, B16, ALU.mult, ALU.add)
                else:
                    nc.scalar.activation(p_t.bitcast(F16)[:], s_ps[:], AF.Exp)

            def emit_pv(p_t, pv_banks, j, start, stop):
                p16 = p_t.bitcast(F16)
                for c in range(4):
                    bank = pv_banks[c // 2]
                    for h in range(2):
                        nc.tensor.matmul(
                            bank[:, (c % 2) * 130 + h * 65:
                                 (c % 2) * 130 + h * 65 + 65],
                            lhsT=p16[:, h * WI + c * 128:h * WI + c * 128 + 128],
                            rhs=v2[:, j, h * 65:h * 65 + 65],
                            start=(start and c % 2 == 0 and h == 0),
                            stop=stop,
                            skip_group_check=True,
                        )

            # ---- deferred epilogue pieces (ctx = prev i-block state) -------
            def emit_epilogue_piece(ctx, piece):
                ib = ctx["ib"]
                i0 = ib * WI
                if piece == 0:  # reciprocal of the 8 Z columns (2 DVE ops)
                    for b in (0, 1):
                        zr = sbsm.tile([128, 4, 1], F32, tag=f"zr{b}",
                                       name="zr")
                        nc.vector.reciprocal(
                            zr[:],
                            ctx["pv"][b][:, 0:260].rearrange(
                                "p (g x) -> p g x", x=65)[:, :, 64:65])
                        ctx[f"zr{b}"] = zr
                elif 1 <= piece <= 4:  # normalize chunk c (2 DVE ops)
                    c = piece - 1
                    onc = sbsm.tile([128, 128], BF16, tag=f"on{c % 2}",
                                    name="onc")
                    b = c // 2
                    for h in (0, 1):
                        nc.vector.tensor_scalar(
                            onc[:, h * 64:h * 64 + 64],
                            ctx["pv"][b][:, (c % 2) * 130 + h * 65:
                                         (c % 2) * 130 + h * 65 + 64],
                            ctx[f"zr{b}"][:, (c % 2) * 2 + h, :],
                            None, ALU.mult)
                    ctx[f"on{c}"] = onc
                elif 5 <= piece <= 8:  # DMA-transpose chunk c
                    c = piece - 5
                    if c == 0:
                        ctx["onT"] = sbsm.tile([128, 4, 128], BF16, tag="onT",
                                               name="onT")
                    nc.sync.dma_start(ctx["onT"][:, c, :], ctx[f"on{c}"][:],
                                      transpose=True)
                elif 9 <= piece <= 12:  # projection chunk o (PE)
                    o = piece - 9
                    pr = ps_v.tile([128, WI], F32,
                                   tag=f"v{o % 2}{ib % 2}", name="pr_ps")
                    nc.tensor.matmul(
                        pr[:], lhsT=wo2_sb[:, o, :],
                        rhs=ctx["onT"][:, :, :],
                        start=True, stop=True,
                    )
                    ctx[f"pr{o}"] = pr
                else:  # pieces 13..16: copy + store chunk o
                    o = piece - 13
                    st = stg.tile([128, WI], F32, tag="st", name="st")
                    nc.vector.tensor_copy(st[:], ctx[f"pr{o}"][:])
                    if small_out:
                        nc.sync.dma_start(out_r[o, :, :], st[:])
                    else:
                        nc.sync.dma_start(out_r[o, :, i0:i0 + WI], st[:])

            # piece -> j-slot within the NEXT i-block
            PIECE_AT = {2: 0, 3: 1, 4: 2, 5: 3, 6: 4, 7: 5, 8: 6, 9: 7,
                        10: 8, 11: 9, 13: 10, 15: 11, 17: 12, 12: 13,
                        14: 14, 19: 15, 21: 16}

            for _rep in range(repeats):
                emit_qk_block(1, k_sb, 0, tag="v11")
                emit_qk_block(0, q_sb, 0, tag="v01")

                epi = {}
                s_tiles = {}
                p_tiles = {}
                pvs = {}
                qps0 = qpsn = None

                def alloc_pv(ib):
                    par = ib % 2
                    banks = (
                        ps_v.tile([128, WI], F32, tag=f"v0{par}", name="pv_a"),
                        ps_v.tile([128, WI], F32, tag=f"v1{par}", name="pv_b"),
                    )
                    pvs[ib] = banks
                    epi[ib] = {"ib": ib, "pv": banks}

                alloc_pv(0)
                s_tiles[0] = emit_scores(0, 0)
                for gs in range(NB * JB):
                    ib, j = divmod(gs, JB)
                    prev = epi.get(ib - 1)
                    if ib == 0:
                        # vt first: its v01 tile must precede alloc_pv(1)'s in
                        # the tag ring, and its DVE copy precede this step's exp
                        emit_vt_chunk(j)
                        if j % 4 == 0 and j // 4 + 1 < NB:
                            emit_qk_block(1, k_sb, j // 4 + 1, tag="v11")
                    # scores one step ahead
                    if gs + 1 < NB * JB:
                        nib, nj = divmod(gs + 1, JB)
                        if nj == 0:
                            alloc_pv(nib)
                        s_tiles[gs + 1] = emit_scores(nib, nj)
                    s_ps = s_tiles.pop(gs)
                    p_t = pp.tile([128, 2 * WI], I16, tag=f"p{gs % 4}",
                                  name="p_t")
                    dve_set = DVE_J_IB0 if ib == 0 else DVE_J
                    emit_exp(s_ps, p_t, j in dve_set)
                    p_tiles[gs] = p_t
                    if ib == 0:
                        if 26 <= j <= 29:
                            qps0 = emit_qk_block(0, q_sb, 1, tag="v11",
                                                 ps=qps0 if j > 26 else None,
                                                 o_only=j - 26,
                                                 copy_on_act=True)
                    elif ib + 1 < NB:
                        if 26 <= j <= 29:
                            qpsn = emit_qk_block(0, q_sb, ib + 1,
                                                 tag=f"v1{1 - ib % 2}",
                                                 ps=qpsn if j > 26 else None,
                                                 o_only=j - 26,
                                                 copy_on_act=True)
                    # PV two steps behind
                    if gs >= 2:
                        pib, pj = divmod(gs - 2, JB)
                        emit_pv(p_tiles.pop(gs - 2), pvs[pib], pj,
                                start=(pj == 0), stop=(pj == JB - 1))
                    if prev is not None and j in PIECE_AT:
                        emit_epilogue_piece(prev, PIECE_AT[j])
                # drain: last two PV steps, then the final epilogue
                for gs in (NB * JB - 2, NB * JB - 1):
                    pib, pj = divmod(gs, JB)
                    emit_pv(p_tiles.pop(gs), pvs[pib], pj,
                            start=(pj == 0), stop=(pj == JB - 1))
                # tail: dependency-interleaved (norm_c -> transpose_c ...)
                for piece in (0, 1, 5, 2, 6, 3, 7, 4, 8,
                              9, 13, 10, 14, 11, 15, 12, 16):
                    emit_epilogue_piece(epi[NB - 1], piece)

    nc.finalize()
    return nc


_PROGRAM_CACHE = {}


def _get_program(**kw) -> bass.Bass:
    key = tuple(sorted(kw.items()))
    if key not in _PROGRAM_CACHE:
        _PROGRAM_CACHE[key] = build_program(**kw)
    return _PROGRAM_CACHE[key]


def _prep_inputs(x, w_qkv, w_out):
    """Build the per-core input maps (all host-side casts)."""
    scale = DIM_HEAD ** -0.5
    xb = x.reshape(2, DIM, N)
    in_maps = []
    for core in range(N_CORES):
        b, hp = divmod(core, 4)
        r0 = hp * 128
        wq = w_qkv[r0:r0 + 128] * scale          # [128, 512]
        wk = w_qkv[DIM + r0:DIM + r0 + 128]      # [128, 512]
        wvr = w_qkv[2 * DIM + r0:2 * DIM + r0 + 128]
        wqk_c = np.concatenate([wq.T, wk.T], axis=1)   # [512, 256]
        wv_t = wvr.T                                   # [512, 128]
        # wo2[hd, o, c] = w_out[o*128 + c, r0 + hd]
        wo2 = w_out[:, r0:r0 + 128].T.reshape(128, 4, 128)
        in_maps.append({
            "x": xb[b].astype(ml_dtypes.bfloat16),
            "wqk": wqk_c.astype(ml_dtypes.bfloat16),
            "wv": wv_t.astype(ml_dtypes.bfloat16),
            "wo2": wo2.astype(ml_dtypes.bfloat16),
        })
    return in_maps


def _run(nc, in_maps):
    try:
        return run_bass_kernel_spmd(nc, in_maps, core_ids=list(range(N_CORES)))
    except Exception:
        # one retry: a previously-wedged device surfaces as a transient
        # NRT_EXEC_UNIT_UNRECOVERABLE on the first execution
        return run_bass_kernel_spmd(nc, in_maps, core_ids=list(range(N_CORES)))


def kernel(x, w_qkv, w_out, b_out):
    nc = _get_program()
    in_maps = _prep_inputs(np.asarray(x), np.asarray(w_qkv), np.asarray(w_out))
    res = _run(nc, in_maps)
    partials = np.stack([r["out"] for r in res.results])  # [8, 512, 4096]
    y = partials.reshape(2, 4, DIM, N).sum(axis=1)
    y += np.asarray(b_out)[None, :, None]
    return y.reshape(2, DIM, 64, 64).astype(np.float32)


# revision 12
# speedup vs baseline: 2.6947x; 2.6947x over previous
"""Trainium2 Bass kernel for nn_Attention_85186381348942.

2D self-attention block: x [2, 512, 64, 64], 8 heads x 64 dim, n = 4096 tokens.
  qkv = w_qkv @ x ; per head: S = (q*scale)^T k ; P = exp(S) (softmax without
  max-subtraction -- logits are small); out = (P/Z) @ v ; y = w_out @ out + b.

Sharding: 8 cores = (batch b in {0,1}) x (head-pair hp in {0..3}); each core
computes 2 heads of one batch and the partial output projection for its head
slice. Host sums the 4 partials per batch and adds bias.

Design (v2): the exp of the 33.5M scores/core is the bottleneck; it is split
across two engines:
 - ACT: direct exp, PSUM fp32 -> SBUF fp16, on ~20 of every 32 j-steps.
 - DVE: one-instruction Schraudolph on the rest: int16(s*2^10/ln2 + b)
   bitcast to fp16 is a ~1.8%-rms exp approximation; softmax normalization
   cancels most of it (end-to-end rel err ~7e-3, gate is 2e-2).
P is fp16 [j=128, A 512 | B 512].  The PV matmul is FLIPPED: per j-step,
8 small matmuls out[i-chunk 128, 65] with lhsT = P chunk (stationary) and
rhs = v' [128, 65] moving (65th column = ones -> Z); 65 moving rows instead
of 512 halves PE busy vs the unflipped form (HW-verified ~209ns/step).
PV accumulators are packed 4-per-PSUM-bank: each bank's first group at j=0
uses start=True (lazy-zeroes the bank); the rest ride start=False on the
pending-zero bytes.
Epilogue per i-block, spread over the next block's j-steps: reciprocal of
the Z columns, per-partition-scalar normalize (out[i,d] layout makes 1/Z a
per-partition scalar), DMA-transpose [128 i, 128 hd] -> [128 hd, 128 i]
(XBAR, on otherwise-idle DMA), then 4 full-K projection matmuls + stores.

PSUM map (8 banks): scores 2 x [128,1024] (tags s) = 4 banks; 4 single-bank
tags v00/v10/v01/v11: PV accumulators of the current i-block parity, while
the opposite parity's banks cycle through epilogue-read -> projection ->
qk-prefetch (and vt/k-stream during ib0).
"""

import numpy as np
import ml_dtypes

import concourse.bass as bass
import concourse.tile as tile
from concourse import bacc, mybir
from concourse.bass_utils import run_bass_kernel_spmd

BF16 = mybir.dt.bfloat16
F16 = mybir.dt.float16
F32 = mybir.dt.float32
I16 = mybir.dt.int16
AF = mybir.ActivationFunctionType
ALU = mybir.AluOpType

HEADS = 8
DIM_HEAD = 64
DIM = 512
N = 4096
N_CORES = 8
NB = 8    # i-blocks
JB = 32   # j-steps per i-block
WI = 512  # i-block width

# fp16 Schraudolph: exp(x) ~ bitcast_i16(int16(x * 2^10/ln2 + (15<<10) - C))
A16 = float(2**10 / np.log(2))
B16 = float(15 * 2**10 - 0.045 * 2**10)

# j-steps whose exp runs on DVE (the rest run on ACT)
DVE_J = frozenset({1, 7, 8, 10, 13, 15, 17, 20, 22, 24, 26, 28})
DVE_J_IB0 = frozenset({5, 7, 10, 13, 15, 18, 21, 23})


def build_program(repeats: int = 1, small_out: bool = False) -> bass.Bass:
    nc = bacc.Bacc(None, target_bir_lowering=False, num_devices=N_CORES)

    x = nc.dram_tensor("x", [DIM, N], BF16, kind="ExternalInput")
    wqk = nc.dram_tensor("wqk", [DIM, 256], BF16, kind="ExternalInput")
    wv = nc.dram_tensor("wv", [DIM, 128], BF16, kind="ExternalInput")
    wo2 = nc.dram_tensor("wo2", [128, 4, 128], BF16, kind="ExternalInput")
    if small_out:
        out = nc.dram_tensor("out", [4, 128, WI], F32, kind="ExternalOutput")
        out_r = out
    else:
        out = nc.dram_tensor("out", [DIM, N], F32, kind="ExternalOutput")
        out_r = out.rearrange("(o p) n -> o p n", p=128)

    x_r = x.rearrange("(o p) n -> p o n", p=128)
    wqk_r = wqk.rearrange("(o p) m -> p o m", p=128)
    wv_r = wv.rearrange("(o p) m -> p o m", p=128)

    with tile.TileContext(nc) as tc:
        with (
            tc.tile_pool(name="singles", bufs=1) as singles,
            tc.tile_pool(name="pp", bufs=1) as pp,      # P tiles (int16)
            tc.tile_pool(name="sbsm", bufs=2) as sbsm,  # small sbuf temps
            tc.tile_pool(name="stg", bufs=4) as stg,    # output staging
            tc.tile_pool(name="ps_s", bufs=2, space="PSUM") as ps_s,  # scores
            tc.tile_pool(name="ps_v", bufs=1, space="PSUM") as ps_v,  # pv/misc
        ):
            # ---- input loads: wqk first (it gates k0), then x block 0 at
            # o-chunk granularity so the k0 matmuls start early; bulk after
            wqk_sb = singles.tile([128, 4, 256], BF16)
            nc.sync.dma_start(wqk_sb[:], wqk_r)
            x_sb = singles.tile([128, 4, N], BF16)
            for o in range(4):
                nc.sync.dma_start(x_sb[:, o, 0:WI], x_r[:, o, 0:WI])
            for nb0 in range(1, 3):
                nc.sync.dma_start(
                    x_sb[:, :, nb0 * WI:(nb0 + 1) * WI],
                    x_r[:, :, nb0 * WI:(nb0 + 1) * WI])
            nc.sync.dma_start(x_sb[:, :, 3 * WI:], x_r[:, :, 3 * WI:])
            wv_sb = singles.tile([128, 4, 128], BF16)
            nc.gpsimd.dma_start(wv_sb[:], wv_r)
            wo2_sb = singles.tile([128, 4, 128], BF16)
            nc.gpsimd.dma_start(wo2_sb[:], wo2[:])

            q_sb = singles.tile([128, N], BF16)
            k_sb = singles.tile([128, N], BF16)
            # v' per j-chunk: [j-part, chunk, h*65 + d]; cols 64/129 = ones
            v2 = singles.tile([128, JB, 130], F16)
            nc.vector.memset(v2[:], 1.0)
            # dummy exp: pull the ACT table load into the x-DMA window
            warm = singles.tile([1, 8], F32)
            nc.vector.memset(warm[:], 0.0)
            nc.scalar.activation(warm[:], warm[:], AF.Exp)

            def emit_qk_block(m, dst, nb, tag, ps=None, o_only=None):
                """dst[:, nb] = wqk[:, m-chunk]^T @ x[:, nb-block]."""
                if ps is None:
                    ps = ps_v.tile([128, WI], F32, tag=tag, name="qk_ps")
                chunks = range(4) if o_only is None else [o_only]
                for o in chunks:
                    nc.tensor.matmul(
                        ps[:],
                        lhsT=wqk_sb[:, o, m * 128:(m + 1) * 128],
                        rhs=x_sb[:, o, nb * WI:(nb + 1) * WI],
                        start=(o == 0),
                        stop=(o == 3),
                        skip_group_check=True,
                    )
                if o_only in (None, 3):
                    nc.vector.tensor_copy(dst[:, nb * WI:(nb + 1) * WI], ps[:])
                return ps

            def emit_vt_chunk(t):
                """v2 chunk t (both heads): vT[n, hd] = x^T @ wv."""
                ps = ps_v.tile([128, 128], F32, tag="v01", name="vt_ps")
                for o in range(4):
                    nc.tensor.matmul(
                        ps[:],
                        lhsT=x_sb[:, o, t * 128:(t + 1) * 128],
                        rhs=wv_sb[:, o, :],
                        start=(o == 0),
                        stop=(o == 3),
                    )
                # strided copy into [d 0:64] and [65:129] (ones cols kept)
                nc.vector.tensor_copy(
                    v2[:, t, 0:130].rearrange("p (h d) -> p h d", h=2)[:, :, 0:64],
                    ps[:].rearrange("p (h d) -> p h d", h=2),
                )

            def emit_scores(ib, j):
                i0 = ib * WI
                j0 = j * 128
                s_ps = ps_s.tile([128, 2 * WI], F32, tag="s", name="s_ps")
                nc.tensor.matmul(
                    s_ps[:, 0:WI],
                    lhsT=k_sb[0:64, j0:j0 + 128],
                    rhs=q_sb[0:64, i0:i0 + WI],
                    start=True, stop=True,
                )
                nc.tensor.matmul(
                    s_ps[:, WI:2 * WI],
                    lhsT=k_sb[64:128, j0:j0 + 128],
                    rhs=q_sb[64:128, i0:i0 + WI],
                    start=True, stop=True,
                )
                return s_ps

            def emit_exp(s_ps, p_t, on_dve):
                if on_dve:
                    nc.vector.tensor_scalar(
                        p_t[:], s_ps[:], A16, B16, ALU.mult, ALU.add)
                else:
                    nc.scalar.activation(p_t.bitcast(F16)[:], s_ps[:], AF.Exp)

            def emit_pv(p_t, pv_banks, j, start, stop):
                p16 = p_t.bitcast(F16)
                for c in range(4):
                    bank = pv_banks[c // 2]
                    for h in range(2):
                        nc.tensor.matmul(
                            bank[:, (c % 2) * 130 + h * 65:
                                 (c % 2) * 130 + h * 65 + 65],
                            lhsT=p16[:, h * WI + c * 128:h * WI + c * 128 + 128],
                            rhs=v2[:, j, h * 65:h * 65 + 65],
                            start=(start and c % 2 == 0 and h == 0),
                            stop=stop,
                            skip_group_check=True,
                        )

            # ---- deferred epilogue pieces (ctx = prev i-block state) -------
            def emit_epilogue_piece(ctx, piece):
                ib = ctx["ib"]
                i0 = ib * WI
                if piece == 0:  # reciprocal of the 8 Z columns (2 DVE ops)
                    for b in (0, 1):
                        zr = sbsm.tile([128, 4, 1], F32, tag=f"zr{b}",
                                       name="zr")
                        nc.vector.reciprocal(
                            zr[:],
                            ctx["pv"][b][:, 0:260].rearrange(
                                "p (g x) -> p g x", x=65)[:, :, 64:65])
                        ctx[f"zr{b}"] = zr
                elif 1 <= piece <= 4:  # normalize chunk c (2 DVE ops)
                    c = piece - 1
                    onc = sbsm.tile([128, 128], BF16, tag=f"on{c % 2}",
                                    name="onc")
                    b = c // 2
                    for h in (0, 1):
                        nc.vector.tensor_scalar(
                            onc[:, h * 64:h * 64 + 64],
                            ctx["pv"][b][:, (c % 2) * 130 + h * 65:
                                         (c % 2) * 130 + h * 65 + 64],
                            ctx[f"zr{b}"][:, (c % 2) * 2 + h, :],
                            None, ALU.mult)
                    ctx[f"on{c}"] = onc
                elif 5 <= piece <= 8:  # DMA-transpose chunk c
                    c = piece - 5
                    if c == 0:
                        ctx["onT"] = sbsm.tile([128, 4, 128], BF16, tag="onT",
                                               name="onT")
                    nc.sync.dma_start(ctx["onT"][:, c, :], ctx[f"on{c}"][:],
                                      transpose=True)
                elif 9 <= piece <= 12:  # projection chunk o (PE)
                    o = piece - 9
                    pr = ps_v.tile([128, WI], F32,
                                   tag=f"v{o % 2}{ib % 2}", name="pr_ps")
                    nc.tensor.matmul(
                        pr[:], lhsT=wo2_sb[:, o, :],
                        rhs=ctx["onT"][:, :, :],
                        start=True, stop=True,
                    )
                    ctx[f"pr{o}"] = pr
                else:  # pieces 13..16: copy + store chunk o
                    o = piece - 13
                    st = stg.tile([128, WI], F32, tag="st", name="st")
                    nc.vector.tensor_copy(st[:], ctx[f"pr{o}"][:])
                    if small_out:
                        nc.sync.dma_start(out_r[o, :, :], st[:])
                    else:
                        nc.sync.dma_start(out_r[o, :, i0:i0 + WI], st[:])

            # piece -> j-slot within the NEXT i-block
            PIECE_AT = {2: 0, 3: 1, 4: 2, 5: 3, 6: 4, 7: 5, 8: 6, 9: 7,
                        10: 8, 11: 9, 13: 10, 15: 11, 17: 12, 12: 13,
                        14: 14, 19: 15, 21: 16}

            for _rep in range(repeats):
                emit_qk_block(1, k_sb, 0, tag="v11")
                emit_qk_block(0, q_sb, 0, tag="v01")

                epi = {}
                s_tiles = {}
                p_tiles = {}
                pvs = {}
                qps0 = qpsn = None

                def alloc_pv(ib):
                    par = ib % 2
                    banks = (
                        ps_v.tile([128, WI], F32, tag=f"v0{par}", name="pv_a"),
                        ps_v.tile([128, WI], F32, tag=f"v1{par}", name="pv_b"),
                    )
                    pvs[ib] = banks
                    epi[ib] = {"ib": ib, "pv": banks}

                alloc_pv(0)
                s_tiles[0] = emit_scores(0, 0)
                for gs in range(NB * JB):
                    ib, j = divmod(gs, JB)
                    prev = epi.get(ib - 1)
                    if ib == 0:
                        # vt first: its v01 tile must precede alloc_pv(1)'s in
                        # the tag ring, and its DVE copy precede this step's exp
                        emit_vt_chunk(j)
                        if j % 4 == 0 and j // 4 + 1 < NB:
                            emit_qk_block(1, k_sb, j // 4 + 1, tag="v11")
                    # scores one step ahead
                    if gs + 1 < NB * JB:
                        nib, nj = divmod(gs + 1, JB)
                        if nj == 0:
                            alloc_pv(nib)
                        s_tiles[gs + 1] = emit_scores(nib, nj)
                    s_ps = s_tiles.pop(gs)
                    p_t = pp.tile([128, 2 * WI], I16, tag=f"p{gs % 4}",
                                  name="p_t")
                    dve_set = DVE_J_IB0 if ib == 0 else DVE_J
                    emit_exp(s_ps, p_t, j in dve_set)
                    p_tiles[gs] = p_t
                    if ib == 0:
                        if 26 <= j <= 29:
                            qps0 = emit_qk_block(0, q_sb, 1, tag="v11",
                                                 ps=qps0 if j > 26 else None,
                                                 o_only=j - 26)
                    elif ib + 1 < NB:
                        if 26 <= j <= 29:
                            qpsn = emit_qk_block(0, q_sb, ib + 1,
                                                 tag=f"v1{1 - ib % 2}",
                                                 ps=qpsn if j > 26 else None,
                                                 o_only=j - 26)
                    # PV two steps behind
                    if gs >= 2:
                        pib, pj = divmod(gs - 2, JB)
                        emit_pv(p_tiles.pop(gs - 2), pvs[pib], pj,
                                start=(pj == 0), stop=(pj == JB - 1))
                    if prev is not None and j in PIECE_AT:
                        emit_epilogue_piece(prev, PIECE_AT[j])
                # drain: last two PV steps, then the final epilogue
                for gs in (NB * JB - 2, NB * JB - 1):
                    pib, pj = divmod(gs, JB)
                    emit_pv(p_tiles.pop(gs), pvs[pib], pj,
                            start=(pj == 0), stop=(pj == JB - 1))
                # tail: dependency-interleaved (norm_c -> transpose_c ...)
                for piece in (0, 1, 5, 2, 6, 3, 7, 4, 8,
                              9, 13, 10, 14, 11, 15, 12, 16):
                    emit_epilogue_piece(epi[NB - 1], piece)

    nc.finalize()
    return nc


_PROGRAM_CACHE = {}


def _get_program(**kw) -> bass.Bass:
    key = tuple(sorted(kw.items()))
    if key not in _PROGRAM_CACHE:
        _PROGRAM_CACHE[key] = build_program(**kw)
    return _PROGRAM_CACHE[key]


def _prep_inputs(x, w_qkv, w_out):
    """Build the per-core input maps (all host-side casts)."""
    scale = DIM_HEAD ** -0.5
    xb = x.reshape(2, DIM, N)
    in_maps = []
    for core in range(N_CORES):
        b, hp = divmod(core, 4)
        r0 = hp * 128
        wq = w_qkv[r0:r0 + 128] * scale          # [128, 512]
        wk = w_qkv[DIM + r0:DIM + r0 + 128]      # [128, 512]
        wvr = w_qkv[2 * DIM + r0:2 * DIM + r0 + 128]
        wqk_c = np.concatenate([wq.T, wk.T], axis=1)   # [512, 256]
        wv_t = wvr.T                                   # [512, 128]
        # wo2[hd, o, c] = w_out[o*128 + c, r0 + hd]
        wo2 = w_out[:, r0:r0 + 128].T.reshape(128, 4, 128)
        in_maps.append({
            "x": xb[b].astype(ml_dtypes.bfloat16),
            "wqk": wqk_c.astype(ml_dtypes.bfloat16),
            "wv": wv_t.astype(ml_dtypes.bfloat16),
            "wo2": wo2.astype(ml_dtypes.bfloat16),
        })
    return in_maps


def _run(nc, in_maps):
    try:
        return run_bass_kernel_spmd(nc, in_maps, core_ids=list(range(N_CORES)))
    except Exception:
        # one retry: a previously-wedged device surfaces as a transient
        # NRT_EXEC_UNIT_UNRECOVERABLE on the first execution
        return run_bass_kernel_spmd(nc, in_maps, core_ids=list(range(N_CORES)))


def kernel(x, w_qkv, w_out, b_out):
    nc = _get_program()
    in_maps = _prep_inputs(np.asarray(x), np.asarray(w_qkv), np.asarray(w_out))
    res = _run(nc, in_maps)
    partials = np.stack([r["out"] for r in res.results])  # [8, 512, 4096]
    y = partials.reshape(2, 4, DIM, N).sum(axis=1)
    y += np.asarray(b_out)[None, :, None]
    return y.reshape(2, DIM, 64, 64).astype(np.float32)
